# revision 11
# baseline (speedup 1.0000x reference)
"""Trainium2 Bass kernel for causal self-attention (B=2, S=2048, D=1024, H=16).

Sharding: 8 cores = 2 (batch) x 4 (head groups of 4 heads) — data parallel on
batch, tensor parallel on heads. Each core computes, for its batch b and its
4 heads (256 of the 1024 model dims):

  qT/kT = Wq_slice^T x^T            transposed layouts [head_dim, seq], fp16
  v     = x Wv_slice                natural layout [seq, head_dim], fp16
  per head pair (2 heads share the 128 partitions):
    scoresT[kv, q] blocks on PE (two row-packed K=64 matmuls),
    exp on ACT (psum -> fp16 sbuf), causal-triangle mask multiply on DVE
    restricted to the single 128-col partial strip of each diagonal block,
    P^T V + replicated ones-row denominators on PE (col-packed M=64),
    normalize: one reciprocal_approx_fast over the merged 2-bank av tile
    + one tensor_mul per partition half.
  oT_partial = Wo_slice^T attnT     [1024, seq] fp16 partial

Host: feeds x^T and weight slices pre-arranged to the on-chip [partition,
contraction-tile, free] layout (so every DMA descriptor is a contiguous
4-8KB per-partition run), sums the 4 fp16 partials per batch (the
"all-reduce" of the o-projection), transposes, adds bo.

All matmuls run in fp16 (1 cyc/row on PE) with fp32 PSUM accumulation;
softmax scale 1/sqrt(64) is folded into Wq on the host. Projections are
interleaved with the attention loop as fillers (including tail fillers after
each kv loop to cover the softmax-normalize latency), and input DMA triggers
are spread across the sync/scalar/gpsimd queues.
"""

import numpy as np

import concourse.bacc as bacc
import concourse.tile as tile
from concourse import mybir
from concourse.bass_utils import run_bass_kernel_spmd

B, S, D, H = 2, 2048, 1024, 16
HD = D // H          # 64
P = 128
NCORES = 8
GROUPS = 4           # head groups (tensor parallel)
HPG = H // GROUPS    # 4 heads per group
CD = HPG * HD        # 256 local head dims per core
QT = 512             # q tile (matmul free dim)
KT = 128             # kv tile (psum partition dim)
NQT = S // QT        # 4
NKT = S // KT        # 16
KD = D // P          # 8 contraction tiles over the model dim

F32 = mybir.dt.float32
F16 = mybir.dt.float16
EXP = mybir.ActivationFunctionType.Exp

_NC_CACHE = {}


def _build_nc():
    if "nc" in _NC_CACHE:
        return _NC_CACHE["nc"]
    nc = bacc.Bacc()
    # all inputs pre-arranged on the host to [partition, ...] layout
    xt = nc.declare_dram_parameter("xt", [P, KD, S], F16, isOutput=False)
    wq = nc.declare_dram_parameter("wq", [P, KD, CD], F16, isOutput=False)
    wk = nc.declare_dram_parameter("wk", [P, KD, CD], F16, isOutput=False)
    wv = nc.declare_dram_parameter("wv", [P, KD, CD], F16, isOutput=False)
    wo = nc.declare_dram_parameter("wo", [P, 2, D], F16, isOutput=False)
    bq = nc.declare_dram_parameter("bq", [P, 2], F32, isOutput=False)
    bk = nc.declare_dram_parameter("bk", [P, 2], F32, isOutput=False)
    bv = nc.declare_dram_parameter("bv", [HPG, HD], F32, isOutput=False)
    msk = nc.declare_dram_parameter("msk", [P, KT], F16, isOutput=False)
    ot = nc.declare_dram_parameter("ot", [D, S], F16, isOutput=True)

    import concourse.bass as bass

    with tile.TileContext(nc) as tc:
        with tc.tile_pool(name="consts", bufs=1) as consts, \
             tc.tile_pool(name="work", bufs=3) as work, \
             tc.tile_pool(name="ps_s", bufs=2, space="PSUM") as ps_s, \
             tc.tile_pool(name="ps_av", bufs=1, space="PSUM") as ps_av, \
             tc.tile_pool(name="ps_po", bufs=2, space="PSUM") as ps_po:

            # ---- constant / persistent SBUF tensors ----
            xt_sb = consts.tile([P, KD, S], F16)
            wq_sb = consts.tile([P, KD, CD], F16)
            wk_sb = consts.tile([P, KD, CD], F16)
            wv_sb = consts.tile([P, KD, CD], F16)
            wo_sb = consts.tile([P, 2, D], F16)
            bq_sb = consts.tile([P, 2], F32)
            bk_sb = consts.tile([P, 2], F32)
            bv_sb = consts.tile([P, HPG, HD], F32)
            msk_sb = consts.tile([P, KT], F16)
            qT_sb = consts.tile([P, 2, S], F16)
            kT_sb = consts.tile([P, 2, S], F16)
            v2_sb = consts.tile([P, NKT, 2, 3 * HD], F16)
            aT_sb = consts.tile([P, 2, NQT, QT], F16)

            # ---- input DMA triggers spread across three queues; every
            # transfer is per-partition contiguous in DRAM ----
            # sync queue: wq (first proj) + first half of xt per-kt
            nc.sync.dma_start(out=wq_sb, in_=wq[:, :, :])
            for kt in range(0, KD // 2):
                nc.sync.dma_start(out=xt_sb[:, kt, :], in_=xt[:, kt, :])
            # scalar queue: wk + second half of xt
            nc.scalar.dma_start(out=wk_sb, in_=wk[:, :, :])
            for kt in range(KD // 2, KD):
                nc.scalar.dma_start(out=xt_sb[:, kt, :], in_=xt[:, kt, :])
            # gpsimd queue: wv + mask + biases + wo (wo needed latest)
            nc.gpsimd.dma_start(out=wv_sb, in_=wv[:, :, :])
            nc.gpsimd.dma_start(out=msk_sb, in_=msk[:, :])
            nc.gpsimd.dma_start(out=bq_sb, in_=bq[:, :])
            nc.gpsimd.dma_start(out=bk_sb, in_=bk[:, :])
            bv_ap = bv[:, :]
            bv_bc = bass.AP(tensor=bv_ap.tensor, offset=bv_ap.offset,
                            ap=[[0, P]] + list(bv_ap.ap))
            nc.gpsimd.dma_start(out=bv_sb, in_=bv_bc)
            nc.gpsimd.dma_start(out=wo_sb, in_=wo[:, :, :])
            nc.vector.memset(v2_sb[:, :, :, HD:2 * HD], 1.0)

            # round-robin home queues for output DMA triggers
            oq = [nc.scalar, nc.gpsimd]
            oq_i = [0]

            def odma(out, in_):
                oq[oq_i[0] % 2].dma_start(out=out, in_=in_)
                oq_i[0] += 1

            # ---- helpers ----
            def proj_qk(w_sb, b_sb, dst, mt, nts):
                pss = [ps_po.tile([P, QT], F32, tag="po", name=f"ps_qk{j}")
                       for j in range(len(nts))]
                for kt in range(KD):
                    lhs = w_sb[:, kt, mt * P:(mt + 1) * P]
                    for j, nt in enumerate(nts):
                        nc.tensor.matmul(
                            pss[j], lhs,
                            xt_sb[:, kt, nt * QT:(nt + 1) * QT],
                            start=(kt == 0), stop=(kt == KD - 1))
                for j, nt in enumerate(nts):
                    nc.vector.tensor_scalar_add(
                        dst[:, mt, nt * QT:(nt + 1) * QT], pss[j],
                        b_sb[:, mt:mt + 1])

            def proj_v(jt0, jt1):
                for jt in range(jt0, jt1):
                    ps = ps_po.tile([P, QT], F32, tag="po", name="ps_v")
                    for kt in range(KD):
                        nc.tensor.matmul(
                            ps[:, :CD], xt_sb[:, kt, jt * P:(jt + 1) * P],
                            wv_sb[:, kt, :],
                            start=(kt == 0), stop=(kt == KD - 1))
                    psh = ps[:, :CD].rearrange("p (h d) -> p h d", h=HPG)
                    # even heads -> cols 0:64, odd heads -> cols 128:192
                    nc.vector.tensor_add(
                        v2_sb[:, jt, :, 0:HD], psh[:, 0::2, :], bv_sb[:, 0::2, :])
                    nc.vector.tensor_add(
                        v2_sb[:, jt, :, 2 * HD:3 * HD], psh[:, 1::2, :],
                        bv_sb[:, 1::2, :])

            def attention(t, g, fillers=(), tail=()):
                n_kv = 4 * (t + 1)
                # merged av tile, one PSUM bank per half:
                # half A (cols 0:QT): rows 0:64 attn h(2g), 64:128 denom h(2g)
                # half B (cols QT:2QT): rows 0:64 denom h(2g+1), 64:128 attn
                av = ps_av.tile([P, 2 * QT], F32, tag="avden", name="av")
                for kv in range(n_kv):
                    if kv < len(fillers) and fillers[kv] is not None:
                        fillers[kv]()
                    r = kv - 4 * t
                    v0 = KT * r if r >= 1 else 0    # first valid q col
                    s = ps_s.tile([P, 2 * QT], F32, tag="s", name="s")
                    for idx in range(2):
                        p0 = 64 * idx
                        nc.tensor.matmul(
                            s[:, idx * QT + v0:(idx + 1) * QT],
                            kT_sb[p0:p0 + 64, g, kv * KT:(kv + 1) * KT],
                            qT_sb[p0:p0 + 64, g, t * QT + v0:(t + 1) * QT],
                            start=True, stop=True)
                    p_t = work.tile([P, 2 * QT], F16, tag="pt", name="p_t")
                    if r < 1:
                        nc.scalar.activation(p_t, s, EXP)
                    else:
                        for idx in range(2):
                            sl = slice(idx * QT + v0, (idx + 1) * QT)
                            nc.scalar.activation(p_t[:, sl], s[:, sl], EXP)
                    if r >= 0:
                        # only the first 128-col strip of a diagonal block is
                        # partially masked; the rest is fully valid
                        for idx in range(2):
                            sl = slice(idx * QT + KT * r, idx * QT + KT * (r + 1))
                            nc.vector.tensor_mul(p_t[:, sl], p_t[:, sl], msk_sb)
                    for idx in range(2):
                        rhs = p_t[:, idx * QT + v0:(idx + 1) * QT]
                        lhsT = v2_sb[:, kv, g, HD * idx:HD * idx + 2 * HD]
                        nc.tensor.matmul(
                            av[:, idx * QT + v0:(idx + 1) * QT], lhsT, rhs,
                            start=(kv == 0), stop=(kv == n_kv - 1))
                # normalize: aT = av / den, denominators shifted across
                # partition halves via a small SBUF->SBUF DMA.  Issued BEFORE
                # the tail fillers so the fillers' DVE ops (bias add / cast)
                # queue behind the reciprocal on the in-order DVE queue, while
                # their matmuls still cover the PE hole during normalize.
                rc = work.tile([P, 2 * QT], F32, tag="rc", name="rc")
                rc2 = work.tile([P, QT], F32, tag="rc2", name="rc2")
                nc.vector.reciprocal_approx_fast(rc, av)
                nc.sync.dma_start(out=rc2[0:64, :], in_=rc[64:128, 0:QT])
                nc.sync.dma_start(out=rc2[64:128, :], in_=rc[0:64, QT:2 * QT])
                nc.vector.tensor_mul(aT_sb[0:64, g, t, :], av[0:64, 0:QT],
                                     rc2[0:64, :])
                nc.vector.tensor_mul(aT_sb[64:128, g, t, :],
                                     av[64:128, QT:2 * QT], rc2[64:128, :])
                for f in tail:
                    f()

            def oproj(t, mts=None):
                for mt_e in (range(D // P) if mts is None else mts):
                    ps = ps_po.tile([P, QT], F32, tag="po", name="ps_o")
                    for g in range(2):
                        nc.tensor.matmul(
                            ps, wo_sb[:, g, mt_e * P:(mt_e + 1) * P],
                            aT_sb[:, g, t, :],
                            start=(g == 0), stop=(g == 1))
                    ot_t = work.tile([P, QT], F16, tag="ot", name="ot_t")
                    nc.vector.tensor_copy(ot_t, ps)
                    odma(ot[mt_e * P:(mt_e + 1) * P, t * QT:(t + 1) * QT],
                         ot_t)

            # ---- filler-interleaved schedule: the PE stream alternates
            # one projection/o-proj chunk per attention kv-iteration so the
            # in-order PE queue never sits on a block of non-attention work
            # while ACT starves; tail fillers cover the normalize latency ----
            def fq(mt, nt):
                return lambda: proj_qk(wq_sb, bq_sb, qT_sb, mt, [nt])

            def fk(mt, nt):
                return lambda: proj_qk(wk_sb, bk_sb, kT_sb, mt, [nt])

            def fv(jt):
                return lambda: proj_v(jt, jt + 1)

            def fo(t, m0):
                return lambda: oproj(t, mts=[m0, m0 + 1])

            # prefix: just enough for att(0,0)
            proj_qk(wq_sb, bq_sb, qT_sb, 0, [0])
            proj_qk(wk_sb, bk_sb, kT_sb, 0, [0])
            proj_v(0, 1)
            attention(0, 0, [fv(1), fv(2), fv(3)], tail=[fq(1, 0)])
            attention(0, 1, [fk(1, 0), fq(0, 1), fk(0, 1)], tail=[fq(1, 1)])
            attention(1, 0, [fk(1, 1), fv(4), fv(5), fv(6),
                             fo(0, 0), fo(0, 2), fo(0, 4), fv(7)],
                      tail=[fo(0, 6)])
            attention(1, 1, [fq(0, 2), fk(0, 2), fq(1, 2)],
                      tail=[fk(1, 2)])
            attention(2, 0, [None, None, fo(1, 0), fo(1, 2), fo(1, 4),
                             None, None, None,
                             fv(8), fv(9), fv(10), fv(11)], tail=[fo(1, 6)])
            attention(2, 1, [fq(0, 3), fk(0, 3), fq(1, 3)], tail=[fk(1, 3)])
            attention(3, 0, [None, None, fo(2, 0), None,
                             None, None, None, None, None, None, None, None,
                             fv(12), fv(13), fv(14), fv(15)],
                      tail=[fo(2, 4)])
            attention(3, 1, [fo(2, 2)], tail=[fo(2, 6)])
            oproj(3)

    nc.compile()
    _NC_CACHE["nc"] = nc
    return nc


def _make_masks():
    # triangle strip mask: valid iff kv-partition p <= q-col c
    pp = np.arange(P)[:, None]
    cc = np.arange(KT)[None, :]
    return (pp <= cc).astype(np.float16)


def _part_major(a, kd):
    # [kd*P, F] -> [P, kd, F] so each partition's DMA rows are contiguous
    return np.ascontiguousarray(
        a.reshape(kd, P, a.shape[1]).transpose(1, 0, 2))


def _in_maps(x, Wq, bq, Wk, bk, Wv, bv, Wo):
    scale = np.float32(1.0 / np.sqrt(HD))
    masks = _make_masks()
    maps = []
    for core in range(NCORES):
        b, g = divmod(core, GROUPS)
        csl = slice(g * CD, (g + 1) * CD)
        maps.append({
            "xt": _part_major(np.asarray(x[b]).T.astype(np.float16), KD),
            "wq": _part_major((np.asarray(Wq[:, csl]) * scale).astype(np.float16), KD),
            "wk": _part_major(np.asarray(Wk[:, csl]).astype(np.float16), KD),
            "wv": _part_major(np.asarray(Wv[:, csl]).astype(np.float16), KD),
            "wo": _part_major(np.asarray(Wo[csl, :]).astype(np.float16), 2),
            "bq": np.ascontiguousarray(
                (np.asarray(bq[csl]) * scale).astype(np.float32).reshape(2, P).T),
            "bk": np.ascontiguousarray(
                np.asarray(bk[csl]).astype(np.float32).reshape(2, P).T),
            "bv": np.ascontiguousarray(bv[csl]).reshape(HPG, HD).astype(np.float32),
            "msk": masks,
        })
    return maps


def kernel_with_results(x, Wq, bq, Wk, bk, Wv, bv, Wo, bo, trace=False):
    nc = _build_nc()
    maps = _in_maps(x, Wq, bq, Wk, bk, Wv, bv, Wo)
    kwargs = {}
    if trace:
        kwargs = dict(trace=True, trace_cores=[0])
    res = run_bass_kernel_spmd(nc, maps, core_ids=list(range(NCORES)), **kwargs)
    out = np.zeros((B, S, D), dtype=np.float32)
    for b in range(B):
        acc = np.zeros((D, S), dtype=np.float32)
        for g in range(GROUPS):
            acc += res.results[b * GROUPS + g]["ot"].astype(np.float32)
        out[b] = acc.T + np.asarray(bo, dtype=np.float32)[None, :]
    return out, res


def kernel(x, Wq, bq, Wk, bk, Wv, bv, Wo, bo):
    out, _ = kernel_with_results(x, Wq, bq, Wk, bk, Wv, bv, Wo, bo, trace=False)
    return out


# revision 13
# speedup vs baseline: 1.0968x; 1.0968x over previous
"""Trainium2 Bass kernel for causal self-attention (B=2, S=2048, D=1024, H=16).

Sharding: 8 cores = 2 (batch) x 4 (head groups of 4 heads) — data parallel on
batch, tensor parallel on heads. Each core computes, for its batch b and its
4 heads (256 of the 1024 model dims):

  qT/kT = Wq_slice^T x^T            transposed layouts [head_dim, seq], fp16
  v     = x Wv_slice                natural layout [seq, head_dim], fp16
  per head pair (2 heads share the 128 partitions):
    scoresT[kv, q] blocks on PE (two row-packed K=64 matmuls),
    exp on ACT (psum -> fp16 sbuf), causal-triangle mask multiply on DVE
    restricted to the single 128-col partial strip of each diagonal block,
    P^T V + replicated ones-row denominators on PE (col-packed M=64),
    normalize: one reciprocal_approx_fast over the merged 2-bank av tile
    + one tensor_mul per partition half.
  oT_partial = Wo_slice^T attnT     [1024, seq] fp16 partial

Host: feeds x^T and weight slices pre-arranged to the on-chip [partition,
contraction-tile, free] layout (so every DMA descriptor is a contiguous
4-8KB per-partition run), sums the 4 fp16 partials per batch (the
"all-reduce" of the o-projection), transposes, adds bo.

All matmuls run in fp16 (1 cyc/row on PE) with fp32 PSUM accumulation;
softmax scale 1/sqrt(64) is folded into Wq on the host. Projections are
interleaved with the attention loop as fillers (including tail fillers after
each kv loop to cover the softmax-normalize latency), and input DMA triggers
are spread across the sync/scalar/gpsimd queues.
"""

import numpy as np

import concourse.bacc as bacc
import concourse.tile as tile
from concourse import mybir
from concourse.bass_utils import run_bass_kernel_spmd

B, S, D, H = 2, 2048, 1024, 16
HD = D // H          # 64
P = 128
NCORES = 8
GROUPS = 4           # head groups (tensor parallel)
HPG = H // GROUPS    # 4 heads per group
CD = HPG * HD        # 256 local head dims per core
QT = 512             # q tile (matmul free dim)
KT = 128             # kv tile (psum partition dim)
NQT = S // QT        # 4
NKT = S // KT        # 16
KD = D // P          # 8 contraction tiles over the model dim

F32 = mybir.dt.float32
F16 = mybir.dt.float16
EXP = mybir.ActivationFunctionType.Exp

_NC_CACHE = {}


def _build_nc():
    if "nc" in _NC_CACHE:
        return _NC_CACHE["nc"]
    nc = bacc.Bacc()
    # all inputs pre-arranged on the host to [partition, ...] layout
    xt = nc.declare_dram_parameter("xt", [P, KD, S], F16, isOutput=False)
    wq = nc.declare_dram_parameter("wq", [P, KD, CD], F16, isOutput=False)
    wk = nc.declare_dram_parameter("wk", [P, KD, CD], F16, isOutput=False)
    wv = nc.declare_dram_parameter("wv", [P, KD, CD], F16, isOutput=False)
    wo = nc.declare_dram_parameter("wo", [P, 2, D], F16, isOutput=False)
    bq = nc.declare_dram_parameter("bq", [P, 2], F32, isOutput=False)
    bk = nc.declare_dram_parameter("bk", [P, 2], F32, isOutput=False)
    bv = nc.declare_dram_parameter("bv", [HPG, HD], F32, isOutput=False)
    msk = nc.declare_dram_parameter("msk", [P, KT], F16, isOutput=False)
    ot = nc.declare_dram_parameter("ot", [D, S], F16, isOutput=True)

    import concourse.bass as bass

    with tile.TileContext(nc) as tc:
        with tc.tile_pool(name="consts", bufs=1) as consts, \
             tc.tile_pool(name="work", bufs=3) as work, \
             tc.tile_pool(name="ps_s", bufs=2, space="PSUM") as ps_s, \
             tc.tile_pool(name="ps_av", bufs=1, space="PSUM") as ps_av, \
             tc.tile_pool(name="ps_po", bufs=2, space="PSUM") as ps_po:

            # ---- constant / persistent SBUF tensors ----
            xt_sb = consts.tile([P, KD, S], F16)
            wq_sb = consts.tile([P, KD, CD], F16)
            wk_sb = consts.tile([P, KD, CD], F16)
            wv_sb = consts.tile([P, KD, CD], F16)
            wo_sb = consts.tile([P, 2, D], F16)
            bq_sb = consts.tile([P, 2], F32)
            bk_sb = consts.tile([P, 2], F32)
            bv_sb = consts.tile([P, HPG, HD], F32)
            msk_sb = consts.tile([P, KT], F16)
            qT_sb = consts.tile([P, 2, S], F16)
            kT_sb = consts.tile([P, 2, S], F16)
            v2_sb = consts.tile([P, NKT, 2, 3 * HD], F16)
            aT_sb = consts.tile([P, 2, NQT, QT], F16)

            # ---- input DMA triggers spread across three queues; every
            # transfer is per-partition contiguous in DRAM ----
            # sync queue: wq (first proj) + first half of xt per-kt
            nc.sync.dma_start(out=wq_sb, in_=wq[:, :, :])
            for kt in range(0, KD // 2):
                nc.sync.dma_start(out=xt_sb[:, kt, :], in_=xt[:, kt, :])
            # scalar queue: wk + second half of xt
            nc.scalar.dma_start(out=wk_sb, in_=wk[:, :, :])
            for kt in range(KD // 2, KD):
                nc.scalar.dma_start(out=xt_sb[:, kt, :], in_=xt[:, kt, :])
            # gpsimd queue: wv + mask + biases + wo (wo needed latest)
            nc.gpsimd.dma_start(out=wv_sb, in_=wv[:, :, :])
            nc.gpsimd.dma_start(out=msk_sb, in_=msk[:, :])
            nc.gpsimd.dma_start(out=bq_sb, in_=bq[:, :])
            nc.gpsimd.dma_start(out=bk_sb, in_=bk[:, :])
            bv_ap = bv[:, :]
            bv_bc = bass.AP(tensor=bv_ap.tensor, offset=bv_ap.offset,
                            ap=[[0, P]] + list(bv_ap.ap))
            nc.gpsimd.dma_start(out=bv_sb, in_=bv_bc)
            nc.gpsimd.dma_start(out=wo_sb, in_=wo[:, :, :])
            nc.vector.memset(v2_sb[:, :, :, HD:2 * HD], 1.0)

            # round-robin home queues for output DMA triggers
            oq = [nc.scalar, nc.gpsimd]
            oq_i = [0]

            def odma(out, in_):
                oq[oq_i[0] % 2].dma_start(out=out, in_=in_)
                oq_i[0] += 1

            # ---- helpers ----
            def proj_qk(w_sb, b_sb, dst, mt, nts):
                pss = [ps_po.tile([P, QT], F32, tag="po", name=f"ps_qk{j}")
                       for j in range(len(nts))]
                for kt in range(KD):
                    lhs = w_sb[:, kt, mt * P:(mt + 1) * P]
                    for j, nt in enumerate(nts):
                        nc.tensor.matmul(
                            pss[j], lhs,
                            xt_sb[:, kt, nt * QT:(nt + 1) * QT],
                            start=(kt == 0), stop=(kt == KD - 1))
                for j, nt in enumerate(nts):
                    nc.vector.tensor_scalar_add(
                        dst[:, mt, nt * QT:(nt + 1) * QT], pss[j],
                        b_sb[:, mt:mt + 1])

            def proj_v(jt0, jt1):
                for jt in range(jt0, jt1):
                    ps = ps_po.tile([P, QT], F32, tag="po", name="ps_v")
                    for kt in range(KD):
                        nc.tensor.matmul(
                            ps[:, :CD], xt_sb[:, kt, jt * P:(jt + 1) * P],
                            wv_sb[:, kt, :],
                            start=(kt == 0), stop=(kt == KD - 1))
                    psh = ps[:, :CD].rearrange("p (h d) -> p h d", h=HPG)
                    # even heads -> cols 0:64, odd heads -> cols 128:192
                    nc.vector.tensor_add(
                        v2_sb[:, jt, :, 0:HD], psh[:, 0::2, :], bv_sb[:, 0::2, :])
                    nc.vector.tensor_add(
                        v2_sb[:, jt, :, 2 * HD:3 * HD], psh[:, 1::2, :],
                        bv_sb[:, 1::2, :])

            def attention(t, g, fillers=(), tail=(), last=False):
                n_kv = 4 * (t + 1)
                # merged av tile, one PSUM bank per half:
                # half A (cols 0:QT): rows 0:64 attn h(2g), 64:128 denom h(2g)
                # half B (cols QT:2QT): rows 0:64 denom h(2g+1), 64:128 attn
                av = ps_av.tile([P, 2 * QT], F32, tag="avden", name="av")
                for kv in range(n_kv):
                    if kv < len(fillers) and fillers[kv] is not None:
                        fillers[kv]()
                    r = kv - 4 * t
                    v0 = KT * r if r >= 1 else 0    # first valid q col
                    s = ps_s.tile([P, 2 * QT], F32, tag="s", name="s")
                    for idx in range(2):
                        p0 = 64 * idx
                        nc.tensor.matmul(
                            s[:, idx * QT + v0:(idx + 1) * QT],
                            kT_sb[p0:p0 + 64, g, kv * KT:(kv + 1) * KT],
                            qT_sb[p0:p0 + 64, g, t * QT + v0:(t + 1) * QT],
                            start=True, stop=True)
                    p_t = work.tile([P, 2 * QT], F16, tag="pt", name="p_t")
                    if r < 1:
                        nc.scalar.activation(p_t, s, EXP)
                    else:
                        for idx in range(2):
                            sl = slice(idx * QT + v0, (idx + 1) * QT)
                            nc.scalar.activation(p_t[:, sl], s[:, sl], EXP)
                    if r >= 0:
                        # only the first 128-col strip of a diagonal block is
                        # partially masked; the rest is fully valid
                        for idx in range(2):
                            sl = slice(idx * QT + KT * r, idx * QT + KT * (r + 1))
                            nc.vector.tensor_mul(p_t[:, sl], p_t[:, sl], msk_sb)
                    for idx in range(2):
                        rhs = p_t[:, idx * QT + v0:(idx + 1) * QT]
                        lhsT = v2_sb[:, kv, g, HD * idx:HD * idx + 2 * HD]
                        nc.tensor.matmul(
                            av[:, idx * QT + v0:(idx + 1) * QT], lhsT, rhs,
                            start=(kv == 0), stop=(kv == n_kv - 1))
                # normalize: aT = av / den, denominators shifted across
                # partition halves via a small SBUF->SBUF DMA.  Issued BEFORE
                # the tail fillers so the fillers' DVE ops (bias add / cast)
                # queue behind the reciprocal on the in-order DVE queue, while
                # their matmuls still cover the PE hole during normalize.
                # Unless this is the last attention, av is first cast to SBUF
                # so its PSUM banks release after one DVE copy instead of
                # after the swap-DMA -> muls chain (which blocked the next
                # attention's first AV matmul for ~2.3us).
                rc = work.tile([P, 2 * QT], F32, tag="rc", name="rc")
                rc2 = work.tile([P, QT], F32, tag="rc2", name="rc2")
                if last:
                    src_t = av
                else:
                    avh = work.tile([P, 2 * QT], F32, tag="avh", name="avh")
                    nc.vector.tensor_copy(avh, av)
                    src_t = avh
                # column-split reciprocals (full partition range) so each
                # swap DMA fires as soon as its half is ready
                nc.vector.reciprocal_approx_fast(rc[:, 0:QT], src_t[:, 0:QT])
                nc.sync.dma_start(out=rc2[0:64, :], in_=rc[64:128, 0:QT])
                nc.vector.reciprocal_approx_fast(rc[:, QT:2 * QT],
                                                 src_t[:, QT:2 * QT])
                nc.sync.dma_start(out=rc2[64:128, :], in_=rc[0:64, QT:2 * QT])
                nc.vector.tensor_mul(aT_sb[0:64, g, t, :], src_t[0:64, 0:QT],
                                     rc2[0:64, :])
                nc.vector.tensor_mul(aT_sb[64:128, g, t, :],
                                     src_t[64:128, QT:2 * QT], rc2[64:128, :])
                for f in tail:
                    f()

            def oproj(t, mts=None):
                for mt_e in (range(D // P) if mts is None else mts):
                    ps = ps_po.tile([P, QT], F32, tag="po", name="ps_o")
                    for g in range(2):
                        nc.tensor.matmul(
                            ps, wo_sb[:, g, mt_e * P:(mt_e + 1) * P],
                            aT_sb[:, g, t, :],
                            start=(g == 0), stop=(g == 1))
                    ot_t = work.tile([P, QT], F16, tag="ot", name="ot_t")
                    nc.vector.tensor_copy(ot_t, ps)
                    odma(ot[mt_e * P:(mt_e + 1) * P, t * QT:(t + 1) * QT],
                         ot_t)

            # ---- filler-interleaved schedule: the PE stream alternates
            # one projection/o-proj chunk per attention kv-iteration so the
            # in-order PE queue never sits on a block of non-attention work
            # while ACT starves; tail fillers cover the normalize latency ----
            def fq(mt, nt):
                return lambda: proj_qk(wq_sb, bq_sb, qT_sb, mt, [nt])

            def fk(mt, nt):
                return lambda: proj_qk(wk_sb, bk_sb, kT_sb, mt, [nt])

            def fv(jt):
                return lambda: proj_v(jt, jt + 1)

            def fo(t, m0):
                return lambda: oproj(t, mts=[m0, m0 + 1])

            # prefix: just enough for att(0,0).  Q/K/V matmuls are
            # interleaved per contraction chunk so each arriving xt chunk
            # feeds three matmuls instead of one during the input-DMA-bound
            # window; V borrows the (still idle) av PSUM banks.
            psq = ps_po.tile([P, QT], F32, tag="po", name="psq")
            psk = ps_po.tile([P, QT], F32, tag="po", name="psk")
            psv = ps_av.tile([P, 2 * QT], F32, tag="avden", name="psv")
            for kt in range(KD):
                st, sp = (kt == 0), (kt == KD - 1)
                nc.tensor.matmul(psq, wq_sb[:, kt, 0:P],
                                 xt_sb[:, kt, 0:QT], start=st, stop=sp)
                nc.tensor.matmul(psk, wk_sb[:, kt, 0:P],
                                 xt_sb[:, kt, 0:QT], start=st, stop=sp)
                nc.tensor.matmul(psv[:, :CD], xt_sb[:, kt, 0:P],
                                 wv_sb[:, kt, :], start=st, stop=sp)
            nc.vector.tensor_scalar_add(qT_sb[:, 0, 0:QT], psq, bq_sb[:, 0:1])
            nc.vector.tensor_scalar_add(kT_sb[:, 0, 0:QT], psk, bk_sb[:, 0:1])
            psh = psv[:, :CD].rearrange("p (h d) -> p h d", h=HPG)
            nc.vector.tensor_add(v2_sb[:, 0, :, 0:HD], psh[:, 0::2, :],
                                 bv_sb[:, 0::2, :])
            nc.vector.tensor_add(v2_sb[:, 0, :, 2 * HD:3 * HD],
                                 psh[:, 1::2, :], bv_sb[:, 1::2, :])
            attention(0, 0, [fv(1), fv(2), fv(3)], tail=[fq(1, 0)])
            attention(0, 1, [fk(1, 0), fq(0, 1), fk(0, 1)], tail=[fq(1, 1)])
            attention(1, 0, [fk(1, 1), fv(4), fv(5), fv(6),
                             fo(0, 0), fo(0, 2), fo(0, 4), fv(7)],
                      tail=[fo(0, 6)])
            attention(1, 1, [fq(0, 2), fk(0, 2), fq(1, 2)],
                      tail=[fk(1, 2)])
            attention(2, 0, [None, None, fo(1, 0), fo(1, 2), fo(1, 4),
                             None, None, None,
                             fv(8), fv(9), fv(10), fv(11)], tail=[fo(1, 6)])
            attention(2, 1, [fq(0, 3), fk(0, 3), fq(1, 3)], tail=[fk(1, 3)])
            attention(3, 0, [None, None, fo(2, 0), None,
                             None, None, None, None, None, None, None, None,
                             fv(12), fv(13), fv(14), fv(15)],
                      tail=[fo(2, 4)])
            attention(3, 1, [fo(2, 2)], tail=[fo(2, 6)], last=True)
            oproj(3)

    nc.compile()
    _NC_CACHE["nc"] = nc
    return nc


def _make_masks():
    # triangle strip mask: valid iff kv-partition p <= q-col c
    pp = np.arange(P)[:, None]
    cc = np.arange(KT)[None, :]
    return (pp <= cc).astype(np.float16)


def _part_major(a, kd):
    # [kd*P, F] -> [P, kd, F] so each partition's DMA rows are contiguous
    return np.ascontiguousarray(
        a.reshape(kd, P, a.shape[1]).transpose(1, 0, 2))


def _in_maps(x, Wq, bq, Wk, bk, Wv, bv, Wo):
    scale = np.float32(1.0 / np.sqrt(HD))
    masks = _make_masks()
    maps = []
    for core in range(NCORES):
        b, g = divmod(core, GROUPS)
        csl = slice(g * CD, (g + 1) * CD)
        maps.append({
            "xt": _part_major(np.asarray(x[b]).T.astype(np.float16), KD),
            "wq": _part_major((np.asarray(Wq[:, csl]) * scale).astype(np.float16), KD),
            "wk": _part_major(np.asarray(Wk[:, csl]).astype(np.float16), KD),
            "wv": _part_major(np.asarray(Wv[:, csl]).astype(np.float16), KD),
            "wo": _part_major(np.asarray(Wo[csl, :]).astype(np.float16), 2),
            "bq": np.ascontiguousarray(
                (np.asarray(bq[csl]) * scale).astype(np.float32).reshape(2, P).T),
            "bk": np.ascontiguousarray(
                np.asarray(bk[csl]).astype(np.float32).reshape(2, P).T),
            "bv": np.ascontiguousarray(bv[csl]).reshape(HPG, HD).astype(np.float32),
            "msk": masks,
        })
    return maps


def kernel_with_results(x, Wq, bq, Wk, bk, Wv, bv, Wo, bo, trace=False):
    nc = _build_nc()
    maps = _in_maps(x, Wq, bq, Wk, bk, Wv, bv, Wo)
    kwargs = {}
    if trace:
        kwargs = dict(trace=True, trace_cores=[0])
    res = run_bass_kernel_spmd(nc, maps, core_ids=list(range(NCORES)), **kwargs)
    out = np.zeros((B, S, D), dtype=np.float32)
    for b in range(B):
        acc = np.zeros((D, S), dtype=np.float32)
        for g in range(GROUPS):
            acc += res.results[b * GROUPS + g]["ot"].astype(np.float32)
        out[b] = acc.T + np.asarray(bo, dtype=np.float32)[None, :]
    return out, res


def kernel(x, Wq, bq, Wk, bk, Wv, bv, Wo, bo):
    out, _ = kernel_with_results(x, Wq, bq, Wk, bk, Wv, bv, Wo, bo, trace=False)
    return out
